# revision 42
# baseline (speedup 1.0000x reference)
"""Trainium2 Bass kernel for a circular-padded 3x3 conv cellular-automaton step.

Computation (per image):
    z   = conv3x3_circular(x, Wc) ;  Wc = w1 @ w_perc  (host-fused, [96,12,3,3])
    h   = relu(z + b1)
    u   = w2 @ h + b2
    um  = (mask > 0.5) * u            (on-chip, fp8 mask: exact for 0/1)
    out = x + um                      (host-side fp32 add)

Mapping (per core, B=16 split 8 ways -> 2 images/core):
  * conv as ONE matmul per image row: K=108 partitions (dj,di,c), all three
    column shifts dj loaded straight from DRAM as 3 base-offset copies of the
    same 3x-row-duplicated read (9x read amplification in bf16, zero on-chip
    copies, zero dep chains).
  * per 4-row supertile: 4 conv matmuls (N=384 -> one 3-bank PSUM z tile),
    relu+bias split ScalarE (3 rows) / VectorE (1 row) -> bf16 ht with a
    ones-row (row 96) so b2 rides the second matmul.
  * second matmul: w2 zero-padded to [97,32], 4 concurrent 32-column
    tile_position matmuls -> one [128, 384] PSUM tile per supertile.
  * mask multiply on VectorE (PSUM fp32 x fp8 mask -> bf16 supertile-layout
    staging), one compact 48-partition DMA out per chunk; host adds x in fp32.
"""

import sys

if "/opt/trn_rl_repo" not in sys.path:
    sys.path.insert(0, "/opt/trn_rl_repo")

from contextlib import ExitStack

import numpy as np
import ml_dtypes

import concourse.bass as bass
import concourse.tile as tile
from concourse import mybir
from concourse.bass_utils import run_bass_kernel_spmd

B, C, H, W = 16, 12, 384, 384
CH = 96                      # hidden channels
NCORES = 8
BLOC = B // NCORES           # images per core
W2 = W + 2                   # circular-padded row length
PADH = H + 4                 # padded rows: 1 top + 3 bottom
CHUNK = 16                   # image rows per processing chunk
ST = 4                       # rows per supertile (one per PE column group)
NCHUNK = H // CHUNK
NST = CHUNK // ST            # supertiles per chunk
XPACK = CHUNK * W            # packed free length per dj copy (rows at stride W)
STP = ST * W                 # packed free length per supertile (1536 = 3 banks)
MTILES = H // ST             # 96 supertile row-groups per image
STW = NST * W                # supertile-layout free length per chunk

_BF16 = mybir.dt.bfloat16
_F32 = mybir.dt.float32
_F8 = mybir.dt.float8e4

# fp8-e4m3 DoubleRow conv path: measured on HW it does NOT speed up the
# matmuls (DoubleRow streams at the same columns/sec as bf16 here), only
# halves the conv-input DMA, while costing rel err 1.9e-2 vs bf16's 3e-3.
# Keep the safe bf16 path.
FP8_CONV = False
_XDT = _F8 if FP8_CONV else _BF16


def _spill_waits(nc):
    """walrus/trn2 here accepts at most ONE sync-wait per instruction; move
    excess waits onto NoOps inserted immediately before, on the same engine."""
    nspill = 0
    for bbwrap in list(nc.bb_map.values()):
        bb = bbwrap.bb
        out = []
        for inst in bb.instructions:
            si = inst.sync_info
            if si is not None and si.on_wait and len(si.on_wait) > 1:
                waits = list(si.on_wait)
                for w in waits[1:]:
                    nop = mybir.InstNoOp(
                        name=nc.get_next_instruction_name(),
                        engine=inst.engine,
                        sync_info=mybir.SyncInfo(on_wait=[w], on_update=[]),
                        bass_nofuse=True,
                    )
                    nc.register_instruction(nop)
                    out.append(nop)
                    nspill += 1
                si.on_wait = waits[:1]
            out.append(inst)
        try:
            bb.instructions = out
        except Exception:
            bb.instructions.clear()
            bb.instructions.extend(out)
    return nspill


def _build_nc(bloc=BLOC, nchunk=NCHUNK):
    nc = bass.Bass()

    # host-prepacked conv input: per (image, chunk), 108 lines
    # (q = (di*3+dj)*12 + c) of 16 packed rows each, fully contiguous
    xq9 = nc.declare_dram_parameter(
        "xq9", [bloc, nchunk, 108, XPACK], _XDT, isOutput=False
    )
    if FP8_CONV:
        wa = nc.declare_dram_parameter("wa", [54, 2, CH], _F8, isOutput=False)
    else:
        wa = nc.declare_dram_parameter("wa", [108, CH], _BF16, isOutput=False)
    w2p = nc.declare_dram_parameter("w2p", [CH + 1, 32], _BF16, isOutput=False)
    b1 = nc.declare_dram_parameter("b1", [CH, 1], _F32, isOutput=False)
    m128 = nc.declare_dram_parameter("m128", [128, MTILES * W], _F8, isOutput=False)
    out = nc.declare_dram_parameter(
        "out", [bloc, nchunk, ST, C, STW], _BF16, isOutput=True
    )

    with tile.TileContext(nc) as tc, ExitStack() as ctx:
        state = _setup(ctx, tc, wa, w2p, b1, m128)
        _loop_body(tc, state, xq9, out, bloc, nchunk)
    _spill_waits(nc)
    return nc


def _setup(ctx, tc, wa, w2p, b1, m128):
    nc = tc.nc

    const = ctx.enter_context(tc.tile_pool(name="const", bufs=1))
    zp = ctx.enter_context(tc.tile_pool(name="z", bufs=2, space="PSUM"))
    up = ctx.enter_context(tc.tile_pool(name="u", bufs=2, space="PSUM"))

    if FP8_CONV:
        wa_sb = const.tile([54, 2, CH], _F8, name="wa_sb")
        nc.sync.dma_start(out=wa_sb, in_=wa[:, :, :])
    else:
        wa_sb = const.tile([108, CH], _BF16, name="wa_sb")
        nc.sync.dma_start(out=wa_sb, in_=wa[:, :])
    w2p_sb = const.tile([CH + 1, 32], _BF16)
    nc.sync.dma_start(out=w2p_sb, in_=w2p[:, :])
    b1_sb = const.tile([CH, 1], _F32)
    nc.sync.dma_start(out=b1_sb, in_=b1[:, :])
    m128_sb = const.tile([128, MTILES * W], _F8)
    nc.sync.dma_start(out=m128_sb, in_=m128[:, :])

    # manually double-buffered tiles (stable addresses):
    #  - ht: constant ones-row (row 96) carries b2 through the second matmul
    #  - xq: conv input, partitions (dj,di,c), straight from DRAM
    #  - ot: masked-update staging in supertile layout
    hts = [
        const.tile([CH + 1, STP], _BF16, name=f"ht{i}", tag=f"ht{i}")
        for i in range(2)
    ]
    if FP8_CONV:
        xqs = [
            const.tile([54, 2, XPACK], _F8, name=f"xqt{i}", tag=f"xqt{i}")
            for i in range(4)
        ]
    else:
        xqs = [
            const.tile([108, XPACK], _BF16, name=f"xqt{i}", tag=f"xqt{i}")
            for i in range(4)
        ]
    ots = [
        const.tile([128, STW], _BF16, name=f"ott{i}", tag=f"ott{i}") for i in range(3)
    ]
    # xq/ot need no memset (fully overwritten before any real read); only the
    # warmup matmul touches xqs[0] column 0, so initialize just that sliver
    if FP8_CONV:
        nc.vector.memset(xqs[0][:, :, 0:1], 0.0)
    else:
        nc.vector.memset(xqs[0][:, 0:1], 0.0)
    for t in hts:
        nc.vector.memset(t, 0.0)
        nc.vector.memset(t[CH : CH + 1, :], 1.0)

    # warmup matmuls: absorb the weight-load DMA waits on the PE clock so the
    # first real matmul of a chunk only waits on its own input DMAs
    zw = zp.tile([CH, STP], _F32, tag="z")
    if FP8_CONV:
        nc.tensor.matmul(
            zw[:, 0:1], wa_sb, xqs[0][:, :, 0:1], start=True, stop=True,
            perf_mode=mybir.MatmulPerfMode.DoubleRow,
        )
    else:
        nc.tensor.matmul(zw[:, 0:1], wa_sb, xqs[0][:, 0:1], start=True, stop=True)
    uw = up.tile([128, W], _F32, tag="u")
    nc.tensor.matmul(
        uw[0:32, 0:1], w2p_sb, hts[0][:, 0:1], start=True, stop=True,
        tile_position=(0, 0),
    )

    return dict(
        zp=zp, up=up,
        wa_sb=wa_sb, w2p_sb=w2p_sb, b1_sb=b1_sb, m128_sb=m128_sb,
        hts=hts, xqs=xqs, ots=ots,
    )


def _loop_body(tc, state, xq9, out, bloc, nchunk):
    nc = tc.nc
    add = mybir.AluOpType.add
    mult = mybir.AluOpType.mult
    amax = mybir.AluOpType.max
    relu = mybir.ActivationFunctionType.Relu
    zp, up = state["zp"], state["up"]
    wa_sb, w2p_sb, b1_sb, m128_sb = (
        state["wa_sb"], state["w2p_sb"], state["b1_sb"], state["m128_sb"],
    )
    hts, xqs, ots = state["hts"], state["xqs"], state["ots"]

    def emit_update(carry):
        """Second matmul + masked staging for a supertile whose relu was
        emitted one iteration earlier (software pipelining: the PE queue is
        in-order, so the next supertile's convs must precede these)."""
        ht, ot, st, tglob, b, chk, last = carry
        u = up.tile([128, W], _F32, tag="u", name="u")
        for j in range(ST):
            nc.tensor.matmul(
                u[32 * j : 32 * j + 32, :],
                w2p_sb,
                ht[:, j * W : j * W + W],
                start=True,
                stop=True,
                tile_position=(0, 32 * j),
            )
        nc.vector.tensor_tensor(
            ot[:, st * W : st * W + W],
            u,
            m128_sb[:, tglob * W : tglob * W + W],
            mult,
        )
        if last:
            # compact store: only the 48 real partitions (4 j-groups x 12 ch)
            for j in range(ST):
                nc.sync.dma_start(
                    out=out[b, chk, j, :, :], in_=ot[32 * j : 32 * j + 12, :]
                )

    nbuf = 0
    ncbuf = 0
    carry = None
    for b in range(bloc):
        for chk in range(nchunk):
            xq = xqs[ncbuf % 4]
            ot = ots[ncbuf % 3]
            ncbuf += 1

            # one DMA: partitions (di*3+dj)*12+c <- the host-prepacked shifted
            # window, 12288 contiguous bytes per partition line. Issued on
            # gpsimd (SWDGE): descriptors spread across SDMA engines by the
            # partition->port map, unlike HWDGE which serializes a whole
            # instruction onto one engine.
            src = bass.AP(
                tensor=xq9,
                offset=(b * nchunk + chk) * 108 * XPACK,
                ap=[[XPACK, 54], [54 * XPACK, 2], [1, XPACK]]
                if FP8_CONV
                else [[XPACK, 108], [1, XPACK]],
            )
            nc.gpsimd.dma_start(out=xq, in_=src)

            for st in range(NST):
                z = zp.tile([CH, STP], _F32, tag="z")
                # k=2 first: VectorE's relu slice [1152:1536] only needs k2,
                # so emitting it first lets that relu (which gates update j3)
                # start a full conv earlier
                for k in (2, 0, 1):
                    w0 = st * STP + k * 512
                    if FP8_CONV:
                        nc.tensor.matmul(
                            z[:, k * 512 : k * 512 + 512],
                            wa_sb,
                            xq[:, :, w0 : w0 + 512],
                            start=True,
                            stop=True,
                            perf_mode=mybir.MatmulPerfMode.DoubleRow,
                        )
                    else:
                        nc.tensor.matmul(
                            z[:, k * 512 : k * 512 + 512],
                            wa_sb,
                            xq[:, w0 : w0 + 512],
                            start=True,
                            stop=True,
                        )

                # relu split: ScalarE covers rows j=0..2 so updates j0-j2 wait
                # only on it; VectorE covers row j=3
                ht = hts[nbuf % 2]
                nc.scalar.activation(
                    out=ht[0:CH, 0:1152], in_=z[:, 0:1152], func=relu, bias=b1_sb
                )
                nc.vector.tensor_scalar(
                    ht[0:CH, 1152:STP], z[:, 1152:STP], b1_sb, 0.0, add, amax
                )

                if carry is not None:
                    emit_update(carry)
                tglob = chk * NST + st
                carry = (ht, ot, st, tglob, b, chk, st == NST - 1)
                nbuf += 1

    emit_update(carry)


_NC_CACHE = {}


def _get_nc():
    if "nc" not in _NC_CACHE:
        _NC_CACHE["nc"] = _build_nc()
    return _NC_CACHE["nc"]


def _prep_weights(w_perc, w1, b1, w2, b2, mask):
    bf16 = ml_dtypes.bfloat16
    f8 = ml_dtypes.float8_e4m3fn
    wc = np.einsum("hp,pcij->hcij", w1, w_perc).astype(np.float32)  # [96,12,3,3]
    # wa[(3*di + dj)*12 + c, h] = wc[h, c, di, dj]
    wdidjc = wc.transpose(2, 3, 1, 0)  # [di, dj, c, h]
    wa = np.ascontiguousarray(wdidjc.reshape(108, CH))
    if FP8_CONV:
        # DoubleRow k-tiling: wa9[p, t, h] = wa[t*54 + p, h]
        f8 = ml_dtypes.float8_e4m3fn
        wa = np.ascontiguousarray(wa.reshape(2, 54, CH).transpose(1, 0, 2)).astype(f8)
    else:
        wa = wa.astype(bf16)
    w2p = np.zeros((CH + 1, 32), np.float32)
    w2p[0:CH, 0:C] = w2.T
    w2p[CH, 0:C] = b2
    w2p = w2p.astype(bf16)
    b1c = np.ascontiguousarray(b1.reshape(CH, 1)).astype(np.float32)

    mbit = (mask > 0.5).astype(np.float32)
    m128 = np.zeros((128, MTILES * W), np.float32)
    for j in range(ST):
        rows = mbit[j::ST, :].reshape(MTILES * W)
        for c in range(C):
            m128[32 * j + c] = rows
    m128 = m128.astype(f8)
    return wa, w2p, b1c, m128


def _prep_xq9(xs, nchunk):
    """Build the 9x-duplicated conv-input layout for one core's image slice:
    xq9[b, chk, (3*di+dj)*12+c, row*W+w]
        = x[b, c, (CHUNK*chk+row+di-1) % H, (w+dj-1) % W]
    (fp8 DoubleRow path reads it as two k-tiles: q = t*54 + p)
    """
    dt = ml_dtypes.float8_e4m3fn if FP8_CONV else ml_dtypes.bfloat16
    bloc = xs.shape[0]
    tmp = np.empty((bloc, C, nchunk, 9, XPACK), dt)
    base = np.arange(nchunk)[:, None] * CHUNK + np.arange(CHUNK)[None, :]
    for dj in range(3):
        xr = np.roll(xs, 1 - dj, axis=3).astype(dt)
        for di in range(3):
            idx = (base + di - 1) % H
            tmp[:, :, :, 3 * di + dj] = xr[:, :, idx, :].reshape(
                bloc, C, nchunk, XPACK
            )
    # -> [b, chk, q=(3*di+dj)*12+c, n]
    out = tmp.transpose(0, 2, 3, 1, 4).reshape(bloc, nchunk, 108, XPACK)
    return np.ascontiguousarray(out)


def _prep_inputs(x, w_perc, w1, b1, w2, b2, mask):
    bf16 = ml_dtypes.bfloat16
    wa, w2p, b1c, m128 = _prep_weights(w_perc, w1, b1, w2, b2, mask)

    in_maps = []
    for core in range(NCORES):
        sl = slice(core * BLOC, (core + 1) * BLOC)
        m = {"wa": wa, "w2p": w2p, "b1": b1c, "m128": m128}
        m["xq9"] = _prep_xq9(x[sl], NCHUNK)
        in_maps.append(m)
    return in_maps


def _unshard_out(x, core_outs):
    full = np.empty((B, C, H, W), np.float32)
    for core, o in enumerate(core_outs):
        o = np.asarray(o, np.float32).reshape(BLOC, NCHUNK, ST, C, NST, W)
        # [b, chk, j, c, s, w] -> [b, c, (chk s j), w]
        o = o.transpose(0, 3, 1, 4, 2, 5).reshape(BLOC, C, H, W)
        full[core * BLOC : (core + 1) * BLOC] = (
            x[core * BLOC : (core + 1) * BLOC] + o
        )
    return full


def kernel(x, w_perc, w1, b1, w2, b2, mask):
    x = np.asarray(x, dtype=np.float32)
    in_maps = _prep_inputs(
        x,
        np.asarray(w_perc, np.float32),
        np.asarray(w1, np.float32),
        np.asarray(b1, np.float32),
        np.asarray(w2, np.float32),
        np.asarray(b2, np.float32),
        np.asarray(mask, np.float32),
    )
    nc = _get_nc()
    res = run_bass_kernel_spmd(nc, in_maps, core_ids=list(range(NCORES)))
    return _unshard_out(x, [r["out"] for r in res.results])


# revision 44
# speedup vs baseline: 1.0212x; 1.0212x over previous
"""Trainium2 Bass kernel for a circular-padded 3x3 conv cellular-automaton step.

Computation (per image):
    z   = conv3x3_circular(x, Wc) ;  Wc = w1 @ w_perc  (host-fused, [96,12,3,3])
    h   = relu(z + b1)
    u   = w2 @ h + b2
    um  = (mask > 0.5) * u            (on-chip, fp8 mask: exact for 0/1)
    out = x + um                      (host-side fp32 add)

Mapping (per core, B=16 split 8 ways -> 2 images/core):
  * conv as ONE matmul per image row: K=108 partitions (dj,di,c), all three
    column shifts dj loaded straight from DRAM as 3 base-offset copies of the
    same 3x-row-duplicated read (9x read amplification in bf16, zero on-chip
    copies, zero dep chains).
  * per 4-row supertile: 4 conv matmuls (N=384 -> one 3-bank PSUM z tile),
    relu+bias split ScalarE (3 rows) / VectorE (1 row) -> bf16 ht with a
    ones-row (row 96) so b2 rides the second matmul.
  * second matmul: w2 zero-padded to [97,32], 4 concurrent 32-column
    tile_position matmuls -> one [128, 384] PSUM tile per supertile.
  * mask multiply on VectorE (PSUM fp32 x fp8 mask -> bf16 supertile-layout
    staging), one compact 48-partition DMA out per chunk; host adds x in fp32.
"""

import sys

if "/opt/trn_rl_repo" not in sys.path:
    sys.path.insert(0, "/opt/trn_rl_repo")

from contextlib import ExitStack

import numpy as np
import ml_dtypes

import concourse.bass as bass
import concourse.tile as tile
from concourse import mybir
from concourse.bass_utils import run_bass_kernel_spmd

B, C, H, W = 16, 12, 384, 384
CH = 96                      # hidden channels
NCORES = 8
BLOC = B // NCORES           # images per core
W2 = W + 2                   # circular-padded row length
PADH = H + 4                 # padded rows: 1 top + 3 bottom
CHUNK = 16                   # image rows per processing chunk
ST = 4                       # rows per supertile (one per PE column group)
NCHUNK = H // CHUNK
NST = CHUNK // ST            # supertiles per chunk
XPACK = CHUNK * W            # packed free length per dj copy (rows at stride W)
STP = ST * W                 # packed free length per supertile (1536 = 3 banks)
MTILES = H // ST             # 96 supertile row-groups per image
STW = NST * W                # supertile-layout free length per chunk

_BF16 = mybir.dt.bfloat16
_F32 = mybir.dt.float32
_F8 = mybir.dt.float8e4

# fp8-e4m3 DoubleRow conv path: measured on HW it does NOT speed up the
# matmuls (DoubleRow streams at the same columns/sec as bf16 here), only
# halves the conv-input DMA, while costing rel err 1.9e-2 vs bf16's 3e-3.
# Keep the safe bf16 path.
FP8_CONV = False
_XDT = _F8 if FP8_CONV else _BF16


def _spill_waits(nc):
    """walrus/trn2 here accepts at most ONE sync-wait per instruction; move
    excess waits onto NoOps inserted immediately before, on the same engine."""
    nspill = 0
    for bbwrap in list(nc.bb_map.values()):
        bb = bbwrap.bb
        out = []
        for inst in bb.instructions:
            si = inst.sync_info
            if si is not None and si.on_wait and len(si.on_wait) > 1:
                waits = list(si.on_wait)
                for w in waits[1:]:
                    nop = mybir.InstNoOp(
                        name=nc.get_next_instruction_name(),
                        engine=inst.engine,
                        sync_info=mybir.SyncInfo(on_wait=[w], on_update=[]),
                        bass_nofuse=True,
                    )
                    nc.register_instruction(nop)
                    out.append(nop)
                    nspill += 1
                si.on_wait = waits[:1]
            out.append(inst)
        try:
            bb.instructions = out
        except Exception:
            bb.instructions.clear()
            bb.instructions.extend(out)
    return nspill


def _build_nc(bloc=BLOC, nchunk=NCHUNK):
    nc = bass.Bass()

    # host-prepacked conv input: per (image, chunk), 108 lines
    # (q = (di*3+dj)*12 + c) of 16 packed rows each, fully contiguous
    xq9 = nc.declare_dram_parameter(
        "xq9", [bloc, nchunk, 108, XPACK], _XDT, isOutput=False
    )
    if FP8_CONV:
        wa = nc.declare_dram_parameter("wa", [54, 2, CH], _F8, isOutput=False)
    else:
        wa = nc.declare_dram_parameter("wa", [108, CH], _BF16, isOutput=False)
    w2p = nc.declare_dram_parameter("w2p", [CH + 1, 32], _BF16, isOutput=False)
    b1 = nc.declare_dram_parameter("b1", [CH, 1], _F32, isOutput=False)
    m128 = nc.declare_dram_parameter("m128", [128, MTILES * W], _F8, isOutput=False)
    out = nc.declare_dram_parameter(
        "out", [bloc, nchunk, ST, C, STW], _BF16, isOutput=True
    )

    with tile.TileContext(nc) as tc, ExitStack() as ctx:
        state = _setup(ctx, tc, wa, w2p, b1, m128)
        _loop_body(tc, state, xq9, out, bloc, nchunk)
    _spill_waits(nc)
    return nc


def _setup(ctx, tc, wa, w2p, b1, m128):
    nc = tc.nc

    const = ctx.enter_context(tc.tile_pool(name="const", bufs=1))
    zp = ctx.enter_context(tc.tile_pool(name="z", bufs=2, space="PSUM"))
    up = ctx.enter_context(tc.tile_pool(name="u", bufs=2, space="PSUM"))

    if FP8_CONV:
        wa_sb = const.tile([54, 2, CH], _F8, name="wa_sb")
        nc.sync.dma_start(out=wa_sb, in_=wa[:, :, :])
    else:
        wa_sb = const.tile([108, CH], _BF16, name="wa_sb")
        nc.sync.dma_start(out=wa_sb, in_=wa[:, :])
    w2p_sb = const.tile([CH + 1, 32], _BF16)
    nc.sync.dma_start(out=w2p_sb, in_=w2p[:, :])
    b1_sb = const.tile([CH, 1], _F32)
    nc.sync.dma_start(out=b1_sb, in_=b1[:, :])
    m128_sb = const.tile([128, MTILES * W], _F8)
    nc.sync.dma_start(out=m128_sb, in_=m128[:, :])

    # manually double-buffered tiles (stable addresses):
    #  - ht: constant ones-row (row 96) carries b2 through the second matmul
    #  - xq: conv input, partitions (dj,di,c), straight from DRAM
    #  - ot: masked-update staging in supertile layout
    hts = [
        const.tile([CH + 1, STP], _BF16, name=f"ht{i}", tag=f"ht{i}")
        for i in range(2)
    ]
    if FP8_CONV:
        xqs = [
            const.tile([54, 2, XPACK], _F8, name=f"xqt{i}", tag=f"xqt{i}")
            for i in range(4)
        ]
    else:
        xqs = [
            const.tile([108, XPACK], _BF16, name=f"xqt{i}", tag=f"xqt{i}")
            for i in range(4)
        ]
    ots = [
        const.tile([128, STW], _BF16, name=f"ott{i}", tag=f"ott{i}") for i in range(3)
    ]
    # xq/ot need no memset (fully overwritten before any real read); only the
    # warmup matmul touches xqs[0] column 0, so initialize just that sliver
    if FP8_CONV:
        nc.vector.memset(xqs[0][:, :, 0:1], 0.0)
    else:
        nc.vector.memset(xqs[0][:, 0:1], 0.0)
    for t in hts:
        nc.vector.memset(t, 0.0)
        nc.vector.memset(t[CH : CH + 1, :], 1.0)

    # warmup matmuls: absorb the weight-load DMA waits on the PE clock so the
    # first real matmul of a chunk only waits on its own input DMAs
    zw = zp.tile([CH, STP], _F32, tag="z")
    if FP8_CONV:
        nc.tensor.matmul(
            zw[:, 0:1], wa_sb, xqs[0][:, :, 0:1], start=True, stop=True,
            perf_mode=mybir.MatmulPerfMode.DoubleRow,
        )
    else:
        nc.tensor.matmul(zw[:, 0:1], wa_sb, xqs[0][:, 0:1], start=True, stop=True)
    uw = up.tile([128, W], _F32, tag="u")
    nc.tensor.matmul(
        uw[0:32, 0:1], w2p_sb, hts[0][:, 0:1], start=True, stop=True,
        tile_position=(0, 0),
    )

    return dict(
        zp=zp, up=up,
        wa_sb=wa_sb, w2p_sb=w2p_sb, b1_sb=b1_sb, m128_sb=m128_sb,
        hts=hts, xqs=xqs, ots=ots,
    )


def _loop_body(tc, state, xq9, out, bloc, nchunk):
    nc = tc.nc
    add = mybir.AluOpType.add
    mult = mybir.AluOpType.mult
    amax = mybir.AluOpType.max
    relu = mybir.ActivationFunctionType.Relu
    zp, up = state["zp"], state["up"]
    wa_sb, w2p_sb, b1_sb, m128_sb = (
        state["wa_sb"], state["w2p_sb"], state["b1_sb"], state["m128_sb"],
    )
    hts, xqs, ots = state["hts"], state["xqs"], state["ots"]

    def emit_update(carry):
        """Second matmul + masked staging for a supertile whose relu was
        emitted one iteration earlier (software pipelining: the PE queue is
        in-order, so the next supertile's convs must precede these)."""
        ht, ot, st, tglob, b, chk, last = carry
        u = up.tile([128, W], _F32, tag="u", name="u")
        for j in range(ST):
            nc.tensor.matmul(
                u[32 * j : 32 * j + 32, :],
                w2p_sb,
                ht[:, j * W : j * W + W],
                start=True,
                stop=True,
                tile_position=(0, 32 * j),
            )
        nc.vector.tensor_tensor(
            ot[:, st * W : st * W + W],
            u,
            m128_sb[:, tglob * W : tglob * W + W],
            mult,
        )
        if last:
            # compact store: only the 48 real partitions (4 j-groups x 12 ch)
            for j in range(ST):
                nc.sync.dma_start(
                    out=out[b, chk, j, :, :], in_=ot[32 * j : 32 * j + 12, :]
                )

    nbuf = 0
    ncbuf = 0
    carry = None
    for b in range(bloc):
        for chk in range(nchunk):
            xq = xqs[ncbuf % 4]
            ot = ots[ncbuf % 3]
            ncbuf += 1

            # one DMA: partitions (di*3+dj)*12+c <- the host-prepacked shifted
            # window, 12288 contiguous bytes per partition line. Issued on
            # gpsimd (SWDGE): descriptors spread across SDMA engines by the
            # partition->port map, unlike HWDGE which serializes a whole
            # instruction onto one engine.
            src = bass.AP(
                tensor=xq9,
                offset=(b * nchunk + chk) * 108 * XPACK,
                ap=[[XPACK, 54], [54 * XPACK, 2], [1, XPACK]]
                if FP8_CONV
                else [[XPACK, 108], [1, XPACK]],
            )
            nc.gpsimd.dma_start(out=xq, in_=src)

            for st in range(NST):
                z = zp.tile([CH, STP], _F32, tag="z")
                for k in range(3):
                    w0 = st * STP + k * 512
                    if FP8_CONV:
                        nc.tensor.matmul(
                            z[:, k * 512 : k * 512 + 512],
                            wa_sb,
                            xq[:, :, w0 : w0 + 512],
                            start=True,
                            stop=True,
                            perf_mode=mybir.MatmulPerfMode.DoubleRow,
                        )
                    else:
                        nc.tensor.matmul(
                            z[:, k * 512 : k * 512 + 512],
                            wa_sb,
                            xq[:, w0 : w0 + 512],
                            start=True,
                            stop=True,
                        )

                # relu entirely on ScalarE: all four updates gate on one
                # instruction, and VectorE keeps only the mask-multiply so its
                # in-order queue never delays the relu chain
                ht = hts[nbuf % 2]
                nc.scalar.activation(
                    out=ht[0:CH, :], in_=z[:, :], func=relu, bias=b1_sb
                )

                if carry is not None:
                    emit_update(carry)
                tglob = chk * NST + st
                carry = (ht, ot, st, tglob, b, chk, st == NST - 1)
                nbuf += 1

    emit_update(carry)


_NC_CACHE = {}


def _get_nc():
    if "nc" not in _NC_CACHE:
        _NC_CACHE["nc"] = _build_nc()
    return _NC_CACHE["nc"]


def _prep_weights(w_perc, w1, b1, w2, b2, mask):
    bf16 = ml_dtypes.bfloat16
    f8 = ml_dtypes.float8_e4m3fn
    wc = np.einsum("hp,pcij->hcij", w1, w_perc).astype(np.float32)  # [96,12,3,3]
    # wa[(3*di + dj)*12 + c, h] = wc[h, c, di, dj]
    wdidjc = wc.transpose(2, 3, 1, 0)  # [di, dj, c, h]
    wa = np.ascontiguousarray(wdidjc.reshape(108, CH))
    if FP8_CONV:
        # DoubleRow k-tiling: wa9[p, t, h] = wa[t*54 + p, h]
        f8 = ml_dtypes.float8_e4m3fn
        wa = np.ascontiguousarray(wa.reshape(2, 54, CH).transpose(1, 0, 2)).astype(f8)
    else:
        wa = wa.astype(bf16)
    w2p = np.zeros((CH + 1, 32), np.float32)
    w2p[0:CH, 0:C] = w2.T
    w2p[CH, 0:C] = b2
    w2p = w2p.astype(bf16)
    b1c = np.ascontiguousarray(b1.reshape(CH, 1)).astype(np.float32)

    mbit = (mask > 0.5).astype(np.float32)
    m128 = np.zeros((128, MTILES * W), np.float32)
    for j in range(ST):
        rows = mbit[j::ST, :].reshape(MTILES * W)
        for c in range(C):
            m128[32 * j + c] = rows
    m128 = m128.astype(f8)
    return wa, w2p, b1c, m128


def _prep_xq9(xs, nchunk):
    """Build the 9x-duplicated conv-input layout for one core's image slice:
    xq9[b, chk, (3*di+dj)*12+c, row*W+w]
        = x[b, c, (CHUNK*chk+row+di-1) % H, (w+dj-1) % W]
    (fp8 DoubleRow path reads it as two k-tiles: q = t*54 + p)
    """
    dt = ml_dtypes.float8_e4m3fn if FP8_CONV else ml_dtypes.bfloat16
    bloc = xs.shape[0]
    tmp = np.empty((bloc, C, nchunk, 9, XPACK), dt)
    base = np.arange(nchunk)[:, None] * CHUNK + np.arange(CHUNK)[None, :]
    for dj in range(3):
        xr = np.roll(xs, 1 - dj, axis=3).astype(dt)
        for di in range(3):
            idx = (base + di - 1) % H
            tmp[:, :, :, 3 * di + dj] = xr[:, :, idx, :].reshape(
                bloc, C, nchunk, XPACK
            )
    # -> [b, chk, q=(3*di+dj)*12+c, n]
    out = tmp.transpose(0, 2, 3, 1, 4).reshape(bloc, nchunk, 108, XPACK)
    return np.ascontiguousarray(out)


def _prep_inputs(x, w_perc, w1, b1, w2, b2, mask):
    bf16 = ml_dtypes.bfloat16
    wa, w2p, b1c, m128 = _prep_weights(w_perc, w1, b1, w2, b2, mask)

    in_maps = []
    for core in range(NCORES):
        sl = slice(core * BLOC, (core + 1) * BLOC)
        m = {"wa": wa, "w2p": w2p, "b1": b1c, "m128": m128}
        m["xq9"] = _prep_xq9(x[sl], NCHUNK)
        in_maps.append(m)
    return in_maps


def _unshard_out(x, core_outs):
    full = np.empty((B, C, H, W), np.float32)
    for core, o in enumerate(core_outs):
        o = np.asarray(o, np.float32).reshape(BLOC, NCHUNK, ST, C, NST, W)
        # [b, chk, j, c, s, w] -> [b, c, (chk s j), w]
        o = o.transpose(0, 3, 1, 4, 2, 5).reshape(BLOC, C, H, W)
        full[core * BLOC : (core + 1) * BLOC] = (
            x[core * BLOC : (core + 1) * BLOC] + o
        )
    return full


def kernel(x, w_perc, w1, b1, w2, b2, mask):
    x = np.asarray(x, dtype=np.float32)
    in_maps = _prep_inputs(
        x,
        np.asarray(w_perc, np.float32),
        np.asarray(w1, np.float32),
        np.asarray(b1, np.float32),
        np.asarray(w2, np.float32),
        np.asarray(b2, np.float32),
        np.asarray(mask, np.float32),
    )
    nc = _get_nc()
    res = run_bass_kernel_spmd(nc, in_maps, core_ids=list(range(NCORES)))
    return _unshard_out(x, [r["out"] for r in res.results])
